# revision 6
# baseline (speedup 1.0000x reference)
"""FLA gated linear attention (chunked) for Trainium2, 8-core SPMD.

Sharding: 8 cores = B(2) x H(4); each core handles one (batch, head) pair:
  - head-sliced q/k/v/g projections + low-rank gate projection (fused on host
    into one [D, DK] matrix),
  - chunked gated linear attention recurrence (superchunks of 256 positions),
  - fused RMSNorm * swish gate,
  - row-parallel output projection producing a [T, D] partial; host sums the
    4 head-partials per batch.

bf16 compute pipeline: all matmul operands bf16 (PE rate identical to fp32r
but DMA bytes halve and DVE ops speed up); gate log-space chain (softplus /
cumsum / exp) stays fp32. k_bar transposes ride the XBAR DMA-transpose unit
(latency hidden by the projection phase); o transposes stay on the PE where
latency is nil. rsqrt and sigmoid avoid ACT-table switches via the DVE pow
ALU. The recurrence is interleaved into the projection loop (superchunks 2q,
2q+1 run right after quarter q) with the output projection lagged one
superchunk so eplilogue latency hides under PE work. Output partials are
stored bf16 and upcast+summed on host.

Self-contained: hardcodes all shapes; host-side work is only sharding/layout.
"""
import sys
sys.path.insert(0, "/opt/trn_rl_repo")

import numpy as np
import ml_dtypes

B, T, D = 2, 2048, 1024
H = 4
DK, DV = 128, 256
SC, NSC = 256, 8          # superchunk size / count
KT = 8                    # 128-row k-tiles over D
TT = 16                   # 128-row t-tiles over T
NORM = 16.0               # gate logit normalizer
EPS = 1e-6

_CACHE = {}


def _build_program():
    import concourse.tile as tile
    from concourse import bacc, hw_specs, mybir

    # Collapse the ACT piecewise-table choice to a single combined
    # Ln+Exp+Copy set so the scheduler never inserts act-table reloads.
    _keep = {"natural_log_exp_and_others"}
    _orig_tables = hw_specs.get_activation_tables("gen3")
    _filtered = {n: (s if n in _keep else set()) for n, s in _orig_tables.items()}
    _saved_fn = bacc.get_activation_tables
    bacc.get_activation_tables = lambda arch: _filtered

    BF = mybir.dt.bfloat16
    F32 = mybir.dt.float32
    AL = mybir.AluOpType
    ACT = mybir.ActivationFunctionType

    nc = bacc.Bacc()

    xt_d = nc.dram_tensor("xt", (D, T), BF, kind="ExternalInput")
    wq_d = nc.dram_tensor("wq", (128, KT * DK), BF, kind="ExternalInput")
    wk_d = nc.dram_tensor("wk", (128, KT * DK), BF, kind="ExternalInput")
    wz_d = nc.dram_tensor("wz", (128, KT * DK), BF, kind="ExternalInput")
    wv_d = nc.dram_tensor("wv", (128, KT * DV), BF, kind="ExternalInput")
    wg_d = nc.dram_tensor("wg", (128, KT * DV), BF, kind="ExternalInput")
    wo_d = nc.dram_tensor("wo", (128, 2 * D), BF, kind="ExternalInput")
    bz_d = nc.dram_tensor("bz", (DK, 1), F32, kind="ExternalInput")
    out_d = nc.dram_tensor("out", (T, D), BF, kind="ExternalOutput")

    with tile.TileContext(nc) as tc:
        with (
            tc.tile_pool(name="consts", bufs=1) as consts,
            tc.tile_pool(name="wpool", bufs=1) as wpool,
            tc.tile_pool(name="persist", bufs=1) as pers,
            tc.tile_pool(name="small", bufs=4) as small,
            tc.tile_pool(name="xt", bufs=2) as xtp,
            tc.tile_pool(name="stage", bufs=3) as stage,
            tc.tile_pool(name="ps_proj", bufs=2, space="PSUM") as psp,
            tc.tile_pool(name="ps_tp", bufs=1, space="PSUM") as pstp,
            tc.tile_pool(name="ps_pa", bufs=1, space="PSUM") as pspa,
            tc.tile_pool(name="ps_po", bufs=1, space="PSUM") as pspo,
            tc.tile_pool(name="ps_pd", bufs=1, space="PSUM") as pspd,
            tc.tile_pool(name="ps_out", bufs=2, space="PSUM") as psout,
        ):
            # ---- constants (scalar ring so they don't delay x/weights) ----
            jj = np.arange(128)[:, None]
            ii = np.arange(SC)[None, :]
            m0_np = (jj <= ii).astype(np.float32)                    # [128,256]
            m1_np = (jj + 128 <= ii[:, 128:]).astype(np.float32)     # [128,128]
            m01_d = nc.inline_tensor(
                np.concatenate([m0_np, m1_np], axis=1), name="m01_c")
            ident_d = nc.inline_tensor(
                np.eye(128, dtype=np.float32).astype(ml_dtypes.bfloat16),
                name="ident_c")
            m01 = consts.tile([128, 384], F32)
            nc.scalar.dma_start(m01, m01_d[:, :])
            m0 = m01[:, 0:256]
            m1 = m01[:, 256:384]
            ident = consts.tile([128, 128], BF)
            nc.scalar.dma_start(ident, ident_d[:, :])
            bz_sb = consts.tile([128, 1], F32)
            nc.scalar.dma_start(bz_sb, bz_d[:, :])
            neginf = consts.tile([128, SC], F32)
            nc.vector.memset(neginf, -3.0e38)

            xt3 = xt_d.rearrange("(k p) t -> p k t", p=128)

            xqs = [xtp.tile([128, KT, 512], BF, tag="xq", name=f"xq{i}")
                   for i in range(4)]
            wq_sb = wpool.tile([128, KT, DK], BF)
            wk_sb = wpool.tile([128, KT, DK], BF)
            wz_sb = wpool.tile([128, KT, DK], BF)
            wv_sb = wpool.tile([128, KT, DV], BF)
            wg_sb = wpool.tile([128, KT, DV], BF)
            wo_sb = wpool.tile([128, 2, D], BF)
            nc.sync.dma_start(wz_sb, wz_d.rearrange("p (k n) -> p k n", k=KT))
            nc.sync.dma_start(xqs[0][:, 0:4, :], xt3[:, 0:4, 0:512])
            nc.sync.dma_start(xqs[0][:, 4:6, :], xt3[:, 4:6, 0:512])
            nc.sync.dma_start(xqs[0][:, 6:8, :], xt3[:, 6:8, 0:512])
            nc.sync.dma_start(wq_sb, wq_d.rearrange("p (k n) -> p k n", k=KT))
            nc.sync.dma_start(wk_sb, wk_d.rearrange("p (k n) -> p k n", k=KT))
            nc.sync.dma_start(wv_sb, wv_d.rearrange("p (k n) -> p k n", k=KT))
            nc.sync.dma_start(wg_sb, wg_d.rearrange("p (k n) -> p k n", k=KT))

            # ---- persistent activations ----
            qg = pers.tile([128, T], BF)       # q^T * exp(G) * scale
            kg = pers.tile([128, T], BF)       # k^T * exp(-G)
            spf = pers.tile([128, T], F32)     # softplus/cumsum, then exp(-G)
            egf = pers.tile([128, T], F32)     # exp(G)
            v2_sb = pers.tile([128, NSC, 512], BF)   # v, 2 t-tiles per SC
            sg2_sb = pers.tile([128, NSC, 512], BF)  # silu(g)
            kbar = pers.tile([128, NSC, 2, 128], BF)  # [t, sc, jt, dk]
            og_sb = pers.tile([128, TT, DV], BF)
            ogt = pers.tile([128, 2, T], BF)   # gated output transposed
            s_ab = pers.tile([128, 2, DV], BF)  # double-buffered state
            spl = pers.tile([128, NSC], F32)
            elast = pers.tile([128, NSC], F32)

            def outproj(s):
                """PE transposes of og(s) + output projection + store."""
                for it in range(2):
                    tt = s * 2 + it
                    xsl = slice(tt * 128, (tt + 1) * 128)
                    ptp = pstp.tile([128, 2, 128], BF, tag="tp")
                    for k2 in range(2):
                        k2sl = slice(k2 * 128, (k2 + 1) * 128)
                        nc.tensor.transpose(ptp[:, k2, :],
                                            og_sb[:, tt, k2sl], ident)
                    nc.vector.tensor_copy(ogt[:, :, xsl], ptp)
                    for nb in range(2):
                        nsl = slice(nb * 512, nb * 512 + 512)
                        pout = psout.tile([128, 512], F32, tag="pout")
                        for k2 in range(2):
                            nc.tensor.matmul(
                                pout, ogt[:, k2, xsl], wo_sb[:, k2, nsl],
                                start=(k2 == 0), stop=(k2 == 1))
                        st = stage.tile([128, 512], BF, tag="st")
                        if nb == 0:
                            nc.vector.tensor_copy(st, pout)
                            nc.sync.dma_start(out_d[xsl, nsl], st)
                        else:
                            nc.scalar.copy(st, pout)
                            nc.scalar.dma_start(out_d[xsl, nsl], st)

            def rec(s):
                """Recurrence superchunk s; also emits outproj(s-1)."""
                ssl = slice(s * SC, (s + 1) * SC)
                s_cur = s_ab[:, s % 2, :]
                s_nxt = s_ab[:, (s + 1) % 2, :]
                # state update first: S_nxt = elast * S_cur + k_bar^T @ v
                # (the DVE update overlaps the pa/po PE work; skipped for the
                # final superchunk - never read)
                if s < NSC - 1:
                    pd_ = pspd.tile([128, DV], F32, tag="pd")
                    nc.tensor.matmul(pd_, kbar[:, s, 0, :], v2_sb[:, s, 0:256],
                                     start=True, stop=False)
                    nc.tensor.matmul(pd_, kbar[:, s, 1, :],
                                     v2_sb[:, s, 256:512],
                                     start=False, stop=True)
                    if s == 0:
                        nc.vector.tensor_copy(s_nxt, pd_)
                    else:
                        nc.vector.scalar_tensor_tensor(
                            out=s_nxt, in0=s_cur, scalar=elast[:, s:s + 1],
                            in1=pd_, op0=AL.mult, op1=AL.add)
                # intra-chunk scores A^T[j, i]; jt=1 only needs i >= 128
                pa = pspa.tile([128, 512], F32, tag="pa")
                jsl0 = slice(s * SC, s * SC + 128)
                jsl1 = slice(s * SC + 128, s * SC + 256)
                nc.tensor.matmul(pa[:, 0:256], kg[:, jsl0], qg[:, ssl],
                                 start=True, stop=True)
                nc.tensor.matmul(pa[:, 384:512], kg[:, jsl1], qg[:, jsl1],
                                 start=True, stop=True)
                am = small.tile([128, 2, SC], BF, tag="am")
                nc.gpsimd.tensor_mul(am[:, 0, :], pa[:, 0:256], m0)
                nc.gpsimd.tensor_mul(am[:, 1, 128:256], pa[:, 384:512], m1)
                # previous superchunk's output projection covers the am / og
                # dependency latency with PE work
                if s > 0:
                    outproj(s - 1)
                # o = qg @ S_prev + tril(A) @ v   (S_prev is 0 for s == 0)
                po = pspo.tile([128, 512], F32, tag="po")
                for it in range(2):
                    osl = slice(it * DV, it * DV + DV)
                    isl = slice(s * SC + it * 128, s * SC + it * 128 + 128)
                    if s > 0:
                        nc.tensor.matmul(po[:, osl], qg[:, isl], s_cur,
                                         start=True, stop=False)
                    for jt in range(it + 1):
                        nc.tensor.matmul(
                            po[:, osl],
                            am[:, jt, it * 128:it * 128 + 128],
                            v2_sb[:, s, jt * 256:jt * 256 + 256],
                            start=(s == 0 and jt == 0), stop=(jt == it))
                # epilogue: rmsnorm * swish-gate (rsqrt via DVE pow ALU;
                # the sqrt(DV) factor is folded into wo on the host)
                for it in range(2):
                    tt = s * 2 + it
                    osl = slice(it * DV, it * DV + DV)
                    scr = small.tile([128, DV], F32, tag="scr")
                    ssq = small.tile([128, 1], F32, tag="ssq")
                    nc.vector.tensor_tensor_reduce(
                        scr, po[:, osl], po[:, osl], 1.0, DV * EPS,
                        AL.mult, AL.add, accum_out=ssq)
                    rstd = small.tile([128, 1], F32, tag="rstd")
                    nc.vector.tensor_scalar(rstd, ssq, -0.5, None, AL.pow)
                    nc.vector.scalar_tensor_tensor(
                        out=og_sb[:, tt, :], in0=po[:, osl], scalar=rstd,
                        in1=sg2_sb[:, s, osl], op0=AL.mult, op1=AL.mult)

            # ====== projections + recurrence, per T-quarter of 512 ======
            for q4 in range(4):
                tsl = slice(q4 * 512, (q4 + 1) * 512)
                xq = xqs[q4]
                if q4 + 1 < 4:
                    nsl4 = slice((q4 + 1) * 512, (q4 + 2) * 512)
                    nc.sync.dma_start(xqs[q4 + 1][:, 0:4, :],
                                      xt3[:, 0:4, nsl4])
                    nc.sync.dma_start(xqs[q4 + 1][:, 4:8, :],
                                      xt3[:, 4:8, nsl4])
                if q4 == 0:
                    nc.sync.dma_start(
                        wo_sb, wo_d.rearrange("p (k n) -> p k n", k=2))
                # gate path (z) first - it feeds the longest chain
                pz = psp.tile([128, 512], F32, tag="pp")
                for k in range(KT):
                    nc.tensor.matmul(pz, wz_sb[:, k, :], xq[:, k, :],
                                     start=(k == 0), stop=(k == KT - 1))
                # sp = softplus(-(z + b)) = ln(1 + exp(-(z + b)))
                sp = spf[:, tsl]
                eg = egf[:, tsl]
                nc.scalar.activation(sp, pz, ACT.Exp, bias=bz_sb, scale=-1.0)
                nc.scalar.activation(sp, sp, ACT.Ln, bias=1.0)
                # per-superchunk cumsum of softplus
                for i2 in range(2):
                    lsl = slice(i2 * SC, (i2 + 1) * SC)
                    nc.vector.tensor_tensor_scan(
                        sp[:, lsl], sp[:, lsl], neginf, 0.0, AL.add, AL.max)
                # SP at superchunk ends, decay factors
                sp3 = sp.rearrange("p (s c) -> p s c", c=SC)
                nc.scalar.copy(
                    spl[:, 2 * q4:2 * q4 + 2].rearrange("p (s o) -> p s o", o=1),
                    sp3[:, 0:2, SC - 1:SC])
                nc.scalar.activation(elast[:, 2 * q4:2 * q4 + 2],
                                     spl[:, 2 * q4:2 * q4 + 2],
                                     ACT.Exp, scale=-1.0 / NORM)
                # eg = exp(G); exp(-G) = 1/eg on DVE (keeps ACT table fixed)
                nc.scalar.activation(eg, sp, ACT.Exp, scale=-1.0 / NORM)
                nc.vector.reciprocal(sp, eg)
                # qg = q^T * exp(G); kg = k^T * exp(-G)  (direct from PSUM)
                pq = psp.tile([128, 512], F32, tag="pp")
                for k in range(KT):
                    nc.tensor.matmul(pq, wq_sb[:, k, :], xq[:, k, :],
                                     start=(k == 0), stop=(k == KT - 1))
                nc.vector.tensor_mul(qg[:, tsl], pq, eg)
                pk = psp.tile([128, 512], F32, tag="pp")
                for k in range(KT):
                    nc.tensor.matmul(pk, wk_sb[:, k, :], xq[:, k, :],
                                     start=(k == 0), stop=(k == KT - 1))
                nc.vector.tensor_mul(kg[:, tsl], pk, sp)
                # k_bar^T = (kg^T * elast) transposed to [t, dk] via XBAR DMA
                for s in (2 * q4, 2 * q4 + 1):
                    if s == NSC - 1:
                        continue
                    ssl = slice(s * SC, (s + 1) * SC)
                    kbt = small.tile([128, SC], BF, tag="kbt")
                    nc.vector.tensor_scalar_mul(kbt, kg[:, ssl],
                                                elast[:, s:s + 1])
                    nc.scalar.dma_start_transpose(kbar[:, s, :, :], kbt)
                # v, g in normal layout, two t-tiles at a time; the
                # recurrence superchunk interleaves between pair blocks so
                # its Pool/DVE chains hide under projection PE work
                def vg_pair(pair):
                    it0 = (pair % 2) * 2
                    pv = psp.tile([128, 512], F32, tag="pp")
                    for half in range(2):
                        xsl = slice((it0 + half) * 128, (it0 + half + 1) * 128)
                        osl = slice(half * 256, (half + 1) * 256)
                        for k in range(KT):
                            nc.tensor.matmul(pv[:, osl], xq[:, k, xsl],
                                             wv_sb[:, k, :],
                                             start=(k == 0), stop=(k == KT - 1))
                    nc.vector.tensor_copy(v2_sb[:, pair, :], pv)
                    pg = psp.tile([128, 512], F32, tag="pp")
                    for half in range(2):
                        xsl = slice((it0 + half) * 128, (it0 + half + 1) * 128)
                        osl = slice(half * 256, (half + 1) * 256)
                        for k in range(KT):
                            nc.tensor.matmul(pg[:, osl], xq[:, k, xsl],
                                             wg_sb[:, k, :],
                                             start=(k == 0), stop=(k == KT - 1))
                    # silu(g) = g * (1 + exp(-g))^-1 ; stays in Exp/Ln table
                    sgs = small.tile([128, 512], F32, tag="sgs")
                    nc.scalar.activation(sgs, pg, ACT.Exp, scale=-1.0)
                    nc.vector.tensor_scalar(sgs, sgs, 1.0, -1.0, AL.add, AL.pow)
                    nc.gpsimd.scalar_tensor_tensor(
                        out=sg2_sb[:, pair, :], in0=sgs, scalar=1.0, in1=pg,
                        op0=AL.mult, op1=AL.mult)

                vg_pair(2 * q4)
                rec(2 * q4)
                vg_pair(2 * q4 + 1)
                rec(2 * q4 + 1)
            outproj(NSC - 1)
    try:
        nc.finalize()
    finally:
        bacc.get_activation_tables = _saved_fn
    return nc


def _get_nc():
    if "nc" not in _CACHE:
        _CACHE["nc"] = _build_program()
    return _CACHE["nc"]


def _sb_layout(w, kt=KT):
    """[kt*128, N] -> [128, kt*N] matching the SBUF [p, k, n] tile layout."""
    n = w.shape[1]
    return np.ascontiguousarray(
        w.reshape(kt, 128, n).transpose(1, 0, 2).reshape(128, kt * n))


def _make_in_maps(x, Wq, Wk, Wv, Wg, Wgk1, Wgk2, bgk2, gnorm_w, Wo):
    f = np.float32
    bf = ml_dtypes.bfloat16
    x = np.asarray(x, f)
    Wq = np.asarray(Wq, f)
    Wk = np.asarray(Wk, f)
    Wv = np.asarray(Wv, f)
    Wg = np.asarray(Wg, f)
    Wgk1 = np.asarray(Wgk1, f)
    Wgk2 = np.asarray(Wgk2, f)
    bgk2 = np.asarray(bgk2, f)
    gnorm_w = np.asarray(gnorm_w, f)
    Wo = np.asarray(Wo, f)

    scale = f(DK) ** f(-0.5)
    wz_full = Wgk1 @ Wgk2                      # [D, KD] fused low-rank gate proj
    in_maps = []
    for c in range(8):
        b, h = c // 4, c % 4
        kd = slice(h * DK, (h + 1) * DK)
        vd = slice(h * DV, (h + 1) * DV)
        # sqrt(DV)=16 from the rmsnorm rsqrt folding lives in wo
        wo = Wo[vd, :] * gnorm_w[:, None] * 16.0
        in_maps.append({
            "xt": np.ascontiguousarray(x[b].T).astype(bf),
            "wq": _sb_layout(Wq[:, kd] * scale).astype(bf),
            "wk": _sb_layout(Wk[:, kd]).astype(bf),
            "wz": _sb_layout(wz_full[:, kd]).astype(bf),
            "wv": _sb_layout(Wv[:, vd]).astype(bf),
            "wg": _sb_layout(Wg[:, vd]).astype(bf),
            "wo": _sb_layout(wo, kt=2).astype(bf),
            "bz": np.ascontiguousarray(-bgk2[kd]).reshape(DK, 1),
        })
    return in_maps


def _run(in_maps, **kwargs):
    from concourse.bass_utils import run_bass_kernel_spmd
    nc = _get_nc()
    return run_bass_kernel_spmd(nc, in_maps, core_ids=list(range(8)), **kwargs)


def _get_exec():
    """Build (once) a reusable 8-core jitted executable around the Bass NEFF.

    Mirrors bass2jax.run_bass_via_pjrt's multi-core path but without buffer
    donation, so repeat kernel() calls reuse the compiled executable instead
    of re-tracing and re-compiling.
    """
    if "exec" in _CACHE:
        return _CACHE["exec"]
    import jax
    import numpy as _np
    from jax.sharding import Mesh, PartitionSpec
    from jax.experimental.shard_map import shard_map
    import concourse.mybir as mybir
    from concourse import bass2jax
    from concourse.bass2jax import _bass_exec_p, partition_id_tensor

    nc = _get_nc()
    n_cores = 8
    bass2jax.install_neuronx_cc_hook()
    partition_name = nc.partition_id_tensor.name if nc.partition_id_tensor else None
    in_names, out_names, out_avals, zero_outs = [], [], [], []
    for alloc in nc.m.functions[0].allocations:
        if not isinstance(alloc, mybir.MemoryLocationSet):
            continue
        name = alloc.memorylocations[0].name
        if alloc.kind == "ExternalInput":
            if name != partition_name:
                in_names.append(name)
        elif alloc.kind == "ExternalOutput":
            out_names.append(name)
            shape = tuple(alloc.tensor_shape)
            dtype = mybir.dt.np(alloc.dtype)
            out_avals.append(jax.core.ShapedArray(shape, dtype))
            zero_outs.append(_np.zeros(shape, dtype))
    n_params = len(in_names)
    all_in_names = list(in_names) + out_names
    if partition_name is not None:
        all_in_names.append(partition_name)

    def _body(*args):
        operands = list(args)
        if partition_name is not None:
            operands.append(partition_id_tensor())
        outs = _bass_exec_p.bind(
            *operands,
            out_avals=tuple(out_avals),
            in_names=tuple(all_in_names),
            out_names=tuple(out_names),
            lowering_input_output_aliases=(),
            sim_require_finite=True,
            sim_require_nnan=True,
            nc=nc,
        )
        return tuple(outs)

    devices = jax.devices()[:n_cores]
    mesh = Mesh(_np.asarray(devices), ("core",))
    in_specs = (PartitionSpec("core"),) * (n_params + len(out_names))
    out_specs = (PartitionSpec("core"),) * len(out_names)
    fn = jax.jit(shard_map(_body, mesh=mesh, in_specs=in_specs,
                           out_specs=out_specs, check_rep=False),
                 keep_unused=True)
    concat_zero = [
        _np.zeros((n_cores * z.shape[0],) + z.shape[1:], z.dtype)
        for z in zero_outs
    ]
    _CACHE["exec"] = (fn, in_names, out_names, concat_zero, n_cores)
    return _CACHE["exec"]


def kernel(x, Wq, Wk, Wv, Wg, Wgk1, Wgk2, bgk2, gnorm_w, Wo):
    import jax
    in_maps = _make_in_maps(x, Wq, Wk, Wv, Wg, Wgk1, Wgk2, bgk2, gnorm_w, Wo)
    fn, in_names, out_names, concat_zero, n_cores = _get_exec()
    concat_in = [
        np.concatenate([np.asarray(in_maps[c][nm]) for c in range(n_cores)],
                       axis=0)
        for nm in in_names
    ]
    outs = fn(*concat_in, *concat_zero)
    jax.block_until_ready(outs)
    o = np.asarray(outs[out_names.index("out")]).reshape(n_cores, T, D)
    o = o.astype(np.float32)
    out = np.zeros((B, T, D), np.float32)
    for c in range(n_cores):
        out[c // 4] += o[c]
    return out


# revision 9
# speedup vs baseline: 1.0229x; 1.0229x over previous
"""FLA gated linear attention (chunked) for Trainium2, 8-core SPMD.

Sharding: 8 cores = B(2) x H(4); each core handles one (batch, head) pair:
  - head-sliced q/k/v/g projections + low-rank gate projection (fused on host
    into one [D, DK] matrix),
  - chunked gated linear attention recurrence (superchunks of 256 positions),
  - fused RMSNorm * swish gate,
  - row-parallel output projection producing a [T, D] partial; host sums the
    4 head-partials per batch.

bf16 compute pipeline: all matmul operands bf16 (PE rate identical to fp32r
but DMA bytes halve and DVE ops speed up); gate log-space chain (softplus /
cumsum / exp) stays fp32. k_bar transposes ride the XBAR DMA-transpose unit
(latency hidden by the projection phase); o transposes stay on the PE where
latency is nil. rsqrt and sigmoid avoid ACT-table switches via the DVE pow
ALU. The recurrence is interleaved into the projection loop (superchunks 2q,
2q+1 run right after quarter q) with the output projection lagged one
superchunk so eplilogue latency hides under PE work. Output partials are
stored bf16 and upcast+summed on host.

Self-contained: hardcodes all shapes; host-side work is only sharding/layout.
"""
import sys
sys.path.insert(0, "/opt/trn_rl_repo")

import numpy as np
import ml_dtypes

B, T, D = 2, 2048, 1024
H = 4
DK, DV = 128, 256
SC, NSC = 256, 8          # superchunk size / count
KT = 8                    # 128-row k-tiles over D
TT = 16                   # 128-row t-tiles over T
NORM = 16.0               # gate logit normalizer
EPS = 1e-6

_CACHE = {}


def _build_program():
    import concourse.tile as tile
    from concourse import bacc, hw_specs, mybir

    # Collapse the ACT piecewise-table choice to a single combined
    # Ln+Exp+Copy set so the scheduler never inserts act-table reloads.
    _keep = {"natural_log_exp_and_others"}
    _orig_tables = hw_specs.get_activation_tables("gen3")
    _filtered = {n: (s if n in _keep else set()) for n, s in _orig_tables.items()}
    _saved_fn = bacc.get_activation_tables
    bacc.get_activation_tables = lambda arch: _filtered

    BF = mybir.dt.bfloat16
    F32 = mybir.dt.float32
    AL = mybir.AluOpType
    ACT = mybir.ActivationFunctionType

    nc = bacc.Bacc()

    xt_d = nc.dram_tensor("xt", (D, T), BF, kind="ExternalInput")
    wq_d = nc.dram_tensor("wq", (128, KT * DK), BF, kind="ExternalInput")
    wk_d = nc.dram_tensor("wk", (128, KT * DK), BF, kind="ExternalInput")
    wz_d = nc.dram_tensor("wz", (128, KT * DK), BF, kind="ExternalInput")
    wv_d = nc.dram_tensor("wv", (128, KT * DV), BF, kind="ExternalInput")
    wg_d = nc.dram_tensor("wg", (128, KT * DV), BF, kind="ExternalInput")
    wo_d = nc.dram_tensor("wo", (128, 2 * D), BF, kind="ExternalInput")
    bz_d = nc.dram_tensor("bz", (DK, 1), F32, kind="ExternalInput")
    out_d = nc.dram_tensor("out", (T, D), BF, kind="ExternalOutput")

    with tile.TileContext(nc) as tc:
        with (
            tc.tile_pool(name="consts", bufs=1) as consts,
            tc.tile_pool(name="wpool", bufs=1) as wpool,
            tc.tile_pool(name="persist", bufs=1) as pers,
            tc.tile_pool(name="small", bufs=4) as small,
            tc.tile_pool(name="xt", bufs=2) as xtp,
            tc.tile_pool(name="stage", bufs=3) as stage,
            tc.tile_pool(name="ps_proj", bufs=2, space="PSUM") as psp,
            tc.tile_pool(name="ps_tp", bufs=1, space="PSUM") as pstp,
            tc.tile_pool(name="ps_pa", bufs=1, space="PSUM") as pspa,
            tc.tile_pool(name="ps_po", bufs=1, space="PSUM") as pspo,
            tc.tile_pool(name="ps_pd", bufs=1, space="PSUM") as pspd,
            tc.tile_pool(name="ps_out", bufs=2, space="PSUM") as psout,
        ):
            # ---- constants (scalar ring so they don't delay x/weights) ----
            jj = np.arange(128)[:, None]
            ii = np.arange(SC)[None, :]
            m0_np = (jj <= ii).astype(np.float32)                    # [128,256]
            m1_np = (jj + 128 <= ii[:, 128:]).astype(np.float32)     # [128,128]
            m01_d = nc.inline_tensor(
                np.concatenate([m0_np, m1_np], axis=1), name="m01_c")
            ident_d = nc.inline_tensor(
                np.eye(128, dtype=np.float32).astype(ml_dtypes.bfloat16),
                name="ident_c")
            m01 = consts.tile([128, 384], F32)
            nc.scalar.dma_start(m01, m01_d[:, :])
            m0 = m01[:, 0:256]
            m1 = m01[:, 256:384]
            ident = consts.tile([128, 128], BF)
            nc.scalar.dma_start(ident, ident_d[:, :])
            bz_sb = consts.tile([128, 1], F32)
            nc.scalar.dma_start(bz_sb, bz_d[:, :])
            neginf = consts.tile([128, SC], F32)
            nc.vector.memset(neginf, -3.0e38)

            xt3 = xt_d.rearrange("(k p) t -> p k t", p=128)

            xqs = [xtp.tile([128, KT, 512], BF, tag="xq", name=f"xq{i}")
                   for i in range(4)]
            wq_sb = wpool.tile([128, KT, DK], BF)
            wk_sb = wpool.tile([128, KT, DK], BF)
            wz_sb = wpool.tile([128, KT, DK], BF)
            wv_sb = wpool.tile([128, KT, DV], BF)
            wg_sb = wpool.tile([128, KT, DV], BF)
            wo_sb = wpool.tile([128, 2, D], BF)
            nc.sync.dma_start(wz_sb, wz_d.rearrange("p (k n) -> p k n", k=KT))
            nc.sync.dma_start(xqs[0][:, 0:4, :], xt3[:, 0:4, 0:512])
            nc.sync.dma_start(xqs[0][:, 4:6, :], xt3[:, 4:6, 0:512])
            nc.sync.dma_start(xqs[0][:, 6:8, :], xt3[:, 6:8, 0:512])
            nc.sync.dma_start(wq_sb, wq_d.rearrange("p (k n) -> p k n", k=KT))
            nc.sync.dma_start(wk_sb, wk_d.rearrange("p (k n) -> p k n", k=KT))
            nc.sync.dma_start(wv_sb, wv_d.rearrange("p (k n) -> p k n", k=KT))
            nc.sync.dma_start(wg_sb, wg_d.rearrange("p (k n) -> p k n", k=KT))

            # ---- persistent activations ----
            qg = pers.tile([128, T], BF)       # q^T * exp(G) * scale
            kg = pers.tile([128, T], BF)       # k^T * exp(-G)
            spf = pers.tile([128, T], F32)     # softplus/cumsum, then exp(-G)
            egf = pers.tile([128, T], F32)     # exp(G)
            v2_sb = pers.tile([128, NSC, 512], BF)   # v, 2 t-tiles per SC
            sg2_sb = pers.tile([128, NSC, 512], BF)  # silu(g)
            kbar = pers.tile([128, NSC, 2, 128], BF)  # [t, sc, jt, dk]
            og_sb = pers.tile([128, TT, DV], BF)
            ogt = pers.tile([128, 2, T], BF)   # gated output transposed
            s_ab = pers.tile([128, 2, DV], BF)  # double-buffered state
            spl = pers.tile([128, NSC], F32)
            elast = pers.tile([128, NSC], F32)

            def outproj(s):
                """PE transposes of og(s) + output projection + store."""
                for it in range(2):
                    tt = s * 2 + it
                    xsl = slice(tt * 128, (tt + 1) * 128)
                    ptp = pstp.tile([128, 2, 128], BF, tag="tp")
                    for k2 in range(2):
                        k2sl = slice(k2 * 128, (k2 + 1) * 128)
                        nc.tensor.transpose(ptp[:, k2, :],
                                            og_sb[:, tt, k2sl], ident)
                    nc.vector.tensor_copy(ogt[:, :, xsl], ptp)
                    for nb in range(2):
                        nsl = slice(nb * 512, nb * 512 + 512)
                        pout = psout.tile([128, 512], F32, tag="pout")
                        for k2 in range(2):
                            nc.tensor.matmul(
                                pout, ogt[:, k2, xsl], wo_sb[:, k2, nsl],
                                start=(k2 == 0), stop=(k2 == 1))
                        st = stage.tile([128, 512], BF, tag="st")
                        if nb == 0:
                            nc.vector.tensor_copy(st, pout)
                            nc.sync.dma_start(out_d[xsl, nsl], st)
                        else:
                            nc.scalar.copy(st, pout)
                            nc.scalar.dma_start(out_d[xsl, nsl], st)

            def rec(s):
                """Recurrence superchunk s; also emits outproj(s-1)."""
                ssl = slice(s * SC, (s + 1) * SC)
                s_cur = s_ab[:, s % 2, :]
                s_nxt = s_ab[:, (s + 1) % 2, :]
                # state update first: S_nxt = elast * S_cur + k_bar^T @ v
                # (the DVE update overlaps the pa/po PE work; skipped for the
                # final superchunk - never read)
                if s < NSC - 1:
                    pd_ = pspd.tile([128, DV], F32, tag="pd")
                    nc.tensor.matmul(pd_, kbar[:, s, 0, :], v2_sb[:, s, 0:256],
                                     start=True, stop=False)
                    nc.tensor.matmul(pd_, kbar[:, s, 1, :],
                                     v2_sb[:, s, 256:512],
                                     start=False, stop=True)
                    if s == 0:
                        nc.vector.tensor_copy(s_nxt, pd_)
                    else:
                        nc.vector.scalar_tensor_tensor(
                            out=s_nxt, in0=s_cur, scalar=elast[:, s:s + 1],
                            in1=pd_, op0=AL.mult, op1=AL.add)
                # intra-chunk scores A^T[j, i]; jt=1 only needs i >= 128
                pa = pspa.tile([128, 512], F32, tag="pa")
                jsl0 = slice(s * SC, s * SC + 128)
                jsl1 = slice(s * SC + 128, s * SC + 256)
                nc.tensor.matmul(pa[:, 0:256], kg[:, jsl0], qg[:, ssl],
                                 start=True, stop=True)
                nc.tensor.matmul(pa[:, 384:512], kg[:, jsl1], qg[:, jsl1],
                                 start=True, stop=True)
                am = small.tile([128, 2, SC], BF, tag="am")
                nc.gpsimd.tensor_mul(am[:, 0, :], pa[:, 0:256], m0)
                nc.gpsimd.tensor_mul(am[:, 1, 128:256], pa[:, 384:512], m1)
                # previous superchunk's output projection covers the am / og
                # dependency latency with PE work
                if s > 0:
                    outproj(s - 1)
                # o = qg @ S_prev + tril(A) @ v   (S_prev is 0 for s == 0)
                po = pspo.tile([128, 512], F32, tag="po")
                for it in range(2):
                    osl = slice(it * DV, it * DV + DV)
                    isl = slice(s * SC + it * 128, s * SC + it * 128 + 128)
                    if s > 0:
                        nc.tensor.matmul(po[:, osl], qg[:, isl], s_cur,
                                         start=True, stop=False)
                    for jt in range(it + 1):
                        nc.tensor.matmul(
                            po[:, osl],
                            am[:, jt, it * 128:it * 128 + 128],
                            v2_sb[:, s, jt * 256:jt * 256 + 256],
                            start=(s == 0 and jt == 0), stop=(jt == it))
                # epilogue: rmsnorm * swish-gate (rsqrt via DVE pow ALU;
                # the sqrt(DV) factor is folded into wo on the host)
                for it in range(2):
                    tt = s * 2 + it
                    osl = slice(it * DV, it * DV + DV)
                    scr = small.tile([128, DV], F32, tag="scr")
                    ssq = small.tile([128, 1], F32, tag="ssq")
                    nc.vector.tensor_tensor_reduce(
                        scr, po[:, osl], po[:, osl], 1.0, DV * EPS,
                        AL.mult, AL.add, accum_out=ssq)
                    rstd = small.tile([128, 1], F32, tag="rstd")
                    nc.vector.tensor_scalar(rstd, ssq, -0.5, None, AL.pow)
                    nc.vector.scalar_tensor_tensor(
                        out=og_sb[:, tt, :], in0=po[:, osl], scalar=rstd,
                        in1=sg2_sb[:, s, osl], op0=AL.mult, op1=AL.mult)

            # ====== projections + recurrence, per T-quarter of 512 ======
            for q4 in range(4):
                tsl = slice(q4 * 512, (q4 + 1) * 512)
                xq = xqs[q4]
                if q4 + 1 < 4:
                    nsl4 = slice((q4 + 1) * 512, (q4 + 2) * 512)
                    nc.sync.dma_start(xqs[q4 + 1][:, 0:4, :],
                                      xt3[:, 0:4, nsl4])
                    nc.sync.dma_start(xqs[q4 + 1][:, 4:8, :],
                                      xt3[:, 4:8, nsl4])
                if q4 == 0:
                    nc.sync.dma_start(
                        wo_sb, wo_d.rearrange("p (k n) -> p k n", k=2))
                # gate path (z) first - it feeds the longest chain
                pz = psp.tile([128, 512], F32, tag="pp")
                for k in range(KT):
                    nc.tensor.matmul(pz, wz_sb[:, k, :], xq[:, k, :],
                                     start=(k == 0), stop=(k == KT - 1))
                # sp = softplus(-(z + b)) = ln(1 + exp(-(z + b)))
                sp = spf[:, tsl]
                eg = egf[:, tsl]
                nc.scalar.activation(sp, pz, ACT.Exp, bias=bz_sb, scale=-1.0)
                nc.scalar.activation(sp, sp, ACT.Ln, bias=1.0)
                # per-superchunk cumsum of softplus
                for i2 in range(2):
                    lsl = slice(i2 * SC, (i2 + 1) * SC)
                    nc.vector.tensor_tensor_scan(
                        sp[:, lsl], sp[:, lsl], neginf, 0.0, AL.add, AL.max)
                # SP at superchunk ends, decay factors
                sp3 = sp.rearrange("p (s c) -> p s c", c=SC)
                nc.scalar.copy(
                    spl[:, 2 * q4:2 * q4 + 2].rearrange("p (s o) -> p s o", o=1),
                    sp3[:, 0:2, SC - 1:SC])
                nc.scalar.activation(elast[:, 2 * q4:2 * q4 + 2],
                                     spl[:, 2 * q4:2 * q4 + 2],
                                     ACT.Exp, scale=-1.0 / NORM)
                # eg = exp(G); exp(-G) = 1/eg on DVE (keeps ACT table fixed)
                nc.scalar.activation(eg, sp, ACT.Exp, scale=-1.0 / NORM)
                nc.vector.reciprocal(sp, eg)
                # qg = q^T * exp(G); kg = k^T * exp(-G)  (direct from PSUM)
                pq = psp.tile([128, 512], F32, tag="pp")
                for k in range(KT):
                    nc.tensor.matmul(pq, wq_sb[:, k, :], xq[:, k, :],
                                     start=(k == 0), stop=(k == KT - 1))
                nc.vector.tensor_mul(qg[:, tsl], pq, eg)
                pk = psp.tile([128, 512], F32, tag="pp")
                for k in range(KT):
                    nc.tensor.matmul(pk, wk_sb[:, k, :], xq[:, k, :],
                                     start=(k == 0), stop=(k == KT - 1))
                nc.vector.tensor_mul(kg[:, tsl], pk, sp)
                # k_bar^T = (kg^T * elast) transposed to [t, dk] on the PE
                # (bf16 transpose = 1 cyc/row; DMA-transpose would queue
                # behind the input loads on the serial DMA device)
                for s in (2 * q4, 2 * q4 + 1):
                    if s == NSC - 1:
                        continue
                    ssl = slice(s * SC, (s + 1) * SC)
                    kbt = small.tile([128, SC], BF, tag="kbt")
                    nc.vector.tensor_scalar_mul(kbt, kg[:, ssl],
                                                elast[:, s:s + 1])
                    ptp = pstp.tile([128, 2, 128], BF, tag="tp")
                    for k2 in range(2):
                        k2sl = slice(k2 * 128, (k2 + 1) * 128)
                        nc.tensor.transpose(ptp[:, k2, :], kbt[:, k2sl], ident)
                    nc.vector.tensor_copy(kbar[:, s, :, :], ptp)
                # v, g in normal layout, two t-tiles at a time; the
                # recurrence superchunk interleaves between pair blocks so
                # its Pool/DVE chains hide under projection PE work
                def vg_pair(pair):
                    it0 = (pair % 2) * 2
                    pv = psp.tile([128, 512], F32, tag="pp")
                    for half in range(2):
                        xsl = slice((it0 + half) * 128, (it0 + half + 1) * 128)
                        osl = slice(half * 256, (half + 1) * 256)
                        for k in range(KT):
                            nc.tensor.matmul(pv[:, osl], xq[:, k, xsl],
                                             wv_sb[:, k, :],
                                             start=(k == 0), stop=(k == KT - 1))
                    nc.vector.tensor_copy(v2_sb[:, pair, :], pv)
                    pg = psp.tile([128, 512], F32, tag="pp")
                    for half in range(2):
                        xsl = slice((it0 + half) * 128, (it0 + half + 1) * 128)
                        osl = slice(half * 256, (half + 1) * 256)
                        for k in range(KT):
                            nc.tensor.matmul(pg[:, osl], xq[:, k, xsl],
                                             wg_sb[:, k, :],
                                             start=(k == 0), stop=(k == KT - 1))
                    # silu(g) = g * (1 + exp(-g))^-1 ; stays in Exp/Ln table
                    sgs = small.tile([128, 512], F32, tag="sgs")
                    nc.scalar.activation(sgs, pg, ACT.Exp, scale=-1.0)
                    nc.vector.tensor_scalar(sgs, sgs, 1.0, -1.0, AL.add, AL.pow)
                    nc.gpsimd.scalar_tensor_tensor(
                        out=sg2_sb[:, pair, :], in0=sgs, scalar=1.0, in1=pg,
                        op0=AL.mult, op1=AL.mult)

                vg_pair(2 * q4)
                rec(2 * q4)
                vg_pair(2 * q4 + 1)
                rec(2 * q4 + 1)
            outproj(NSC - 1)
    try:
        nc.finalize()
    finally:
        bacc.get_activation_tables = _saved_fn
    return nc


def _get_nc():
    if "nc" not in _CACHE:
        _CACHE["nc"] = _build_program()
    return _CACHE["nc"]


def _sb_layout(w, kt=KT):
    """[kt*128, N] -> [128, kt*N] matching the SBUF [p, k, n] tile layout."""
    n = w.shape[1]
    return np.ascontiguousarray(
        w.reshape(kt, 128, n).transpose(1, 0, 2).reshape(128, kt * n))


def _make_in_maps(x, Wq, Wk, Wv, Wg, Wgk1, Wgk2, bgk2, gnorm_w, Wo):
    f = np.float32
    bf = ml_dtypes.bfloat16
    x = np.asarray(x, f)
    Wq = np.asarray(Wq, f)
    Wk = np.asarray(Wk, f)
    Wv = np.asarray(Wv, f)
    Wg = np.asarray(Wg, f)
    Wgk1 = np.asarray(Wgk1, f)
    Wgk2 = np.asarray(Wgk2, f)
    bgk2 = np.asarray(bgk2, f)
    gnorm_w = np.asarray(gnorm_w, f)
    Wo = np.asarray(Wo, f)

    scale = f(DK) ** f(-0.5)
    wz_full = Wgk1 @ Wgk2                      # [D, KD] fused low-rank gate proj
    in_maps = []
    for c in range(8):
        b, h = c // 4, c % 4
        kd = slice(h * DK, (h + 1) * DK)
        vd = slice(h * DV, (h + 1) * DV)
        # sqrt(DV)=16 from the rmsnorm rsqrt folding lives in wo
        wo = Wo[vd, :] * gnorm_w[:, None] * 16.0
        in_maps.append({
            "xt": np.ascontiguousarray(x[b].T).astype(bf),
            "wq": _sb_layout(Wq[:, kd] * scale).astype(bf),
            "wk": _sb_layout(Wk[:, kd]).astype(bf),
            "wz": _sb_layout(wz_full[:, kd]).astype(bf),
            "wv": _sb_layout(Wv[:, vd]).astype(bf),
            "wg": _sb_layout(Wg[:, vd]).astype(bf),
            "wo": _sb_layout(wo, kt=2).astype(bf),
            "bz": np.ascontiguousarray(-bgk2[kd]).reshape(DK, 1),
        })
    return in_maps


def _run(in_maps, **kwargs):
    from concourse.bass_utils import run_bass_kernel_spmd
    nc = _get_nc()
    return run_bass_kernel_spmd(nc, in_maps, core_ids=list(range(8)), **kwargs)


def _get_exec():
    """Build (once) a reusable 8-core jitted executable around the Bass NEFF.

    Mirrors bass2jax.run_bass_via_pjrt's multi-core path but without buffer
    donation, so repeat kernel() calls reuse the compiled executable instead
    of re-tracing and re-compiling.
    """
    if "exec" in _CACHE:
        return _CACHE["exec"]
    import jax
    import numpy as _np
    from jax.sharding import Mesh, PartitionSpec
    from jax.experimental.shard_map import shard_map
    import concourse.mybir as mybir
    from concourse import bass2jax
    from concourse.bass2jax import _bass_exec_p, partition_id_tensor

    nc = _get_nc()
    n_cores = 8
    bass2jax.install_neuronx_cc_hook()
    partition_name = nc.partition_id_tensor.name if nc.partition_id_tensor else None
    in_names, out_names, out_avals, zero_outs = [], [], [], []
    for alloc in nc.m.functions[0].allocations:
        if not isinstance(alloc, mybir.MemoryLocationSet):
            continue
        name = alloc.memorylocations[0].name
        if alloc.kind == "ExternalInput":
            if name != partition_name:
                in_names.append(name)
        elif alloc.kind == "ExternalOutput":
            out_names.append(name)
            shape = tuple(alloc.tensor_shape)
            dtype = mybir.dt.np(alloc.dtype)
            out_avals.append(jax.core.ShapedArray(shape, dtype))
            zero_outs.append(_np.zeros(shape, dtype))
    n_params = len(in_names)
    all_in_names = list(in_names) + out_names
    if partition_name is not None:
        all_in_names.append(partition_name)

    def _body(*args):
        operands = list(args)
        if partition_name is not None:
            operands.append(partition_id_tensor())
        outs = _bass_exec_p.bind(
            *operands,
            out_avals=tuple(out_avals),
            in_names=tuple(all_in_names),
            out_names=tuple(out_names),
            lowering_input_output_aliases=(),
            sim_require_finite=True,
            sim_require_nnan=True,
            nc=nc,
        )
        return tuple(outs)

    devices = jax.devices()[:n_cores]
    mesh = Mesh(_np.asarray(devices), ("core",))
    in_specs = (PartitionSpec("core"),) * (n_params + len(out_names))
    out_specs = (PartitionSpec("core"),) * len(out_names)
    fn = jax.jit(shard_map(_body, mesh=mesh, in_specs=in_specs,
                           out_specs=out_specs, check_rep=False),
                 keep_unused=True)
    concat_zero = [
        _np.zeros((n_cores * z.shape[0],) + z.shape[1:], z.dtype)
        for z in zero_outs
    ]
    _CACHE["exec"] = (fn, in_names, out_names, concat_zero, n_cores)
    return _CACHE["exec"]


def kernel(x, Wq, Wk, Wv, Wg, Wgk1, Wgk2, bgk2, gnorm_w, Wo):
    import jax
    in_maps = _make_in_maps(x, Wq, Wk, Wv, Wg, Wgk1, Wgk2, bgk2, gnorm_w, Wo)
    fn, in_names, out_names, concat_zero, n_cores = _get_exec()
    concat_in = [
        np.concatenate([np.asarray(in_maps[c][nm]) for c in range(n_cores)],
                       axis=0)
        for nm in in_names
    ]
    outs = fn(*concat_in, *concat_zero)
    jax.block_until_ready(outs)
    o = np.asarray(outs[out_names.index("out")]).reshape(n_cores, T, D)
    o = o.astype(np.float32)
    out = np.zeros((B, T, D), np.float32)
    for c in range(n_cores):
        out[c // 4] += o[c]
    return out


# revision 12
# speedup vs baseline: 1.0829x; 1.0587x over previous
"""FLA gated linear attention (chunked) for Trainium2, 8-core SPMD.

Sharding: 8 cores = B(2) x H(4); each core handles one (batch, head) pair:
  - head-sliced q/k/v/g projections + low-rank gate projection (fused on host
    into one [D, DK] matrix),
  - chunked gated linear attention recurrence (superchunks of 256 positions),
  - fused RMSNorm * swish gate,
  - row-parallel output projection producing a [T, D] partial; host sums the
    4 head-partials per batch.

bf16 compute pipeline: all matmul operands bf16 (PE rate identical to fp32r
but DMA bytes halve and DVE ops speed up); gate log-space chain (softplus /
cumsum / exp) stays fp32. k_bar transposes ride the XBAR DMA-transpose unit
(latency hidden by the projection phase); o transposes stay on the PE where
latency is nil. rsqrt and sigmoid avoid ACT-table switches via the DVE pow
ALU. The recurrence is interleaved into the projection loop (superchunks 2q,
2q+1 run right after quarter q) with the output projection lagged one
superchunk so eplilogue latency hides under PE work. Output partials are
stored bf16 and upcast+summed on host.

Self-contained: hardcodes all shapes; host-side work is only sharding/layout.
"""
import sys
sys.path.insert(0, "/opt/trn_rl_repo")

import numpy as np
import ml_dtypes

B, T, D = 2, 2048, 1024
H = 4
DK, DV = 128, 256
SC, NSC = 256, 8          # superchunk size / count
KT = 8                    # 128-row k-tiles over D
TT = 16                   # 128-row t-tiles over T
NORM = 16.0               # gate logit normalizer
EPS = 1e-6

_CACHE = {}


def _build_program():
    import concourse.tile as tile
    from concourse import bacc, hw_specs, mybir

    # Collapse the ACT piecewise-table choice to a single combined
    # Ln+Exp+Copy set so the scheduler never inserts act-table reloads.
    _keep = {"natural_log_exp_and_others"}
    _orig_tables = hw_specs.get_activation_tables("gen3")
    _filtered = {n: (s if n in _keep else set()) for n, s in _orig_tables.items()}
    _saved_fn = bacc.get_activation_tables
    bacc.get_activation_tables = lambda arch: _filtered

    BF = mybir.dt.bfloat16
    F32 = mybir.dt.float32
    AL = mybir.AluOpType
    ACT = mybir.ActivationFunctionType

    nc = bacc.Bacc()

    xt_d = nc.dram_tensor("xt", (D, T), BF, kind="ExternalInput")
    wq_d = nc.dram_tensor("wq", (128, KT * DK), BF, kind="ExternalInput")
    wk_d = nc.dram_tensor("wk", (128, KT * DK), BF, kind="ExternalInput")
    wz_d = nc.dram_tensor("wz", (128, KT * DK), BF, kind="ExternalInput")
    wv_d = nc.dram_tensor("wv", (128, KT * DV), BF, kind="ExternalInput")
    wg_d = nc.dram_tensor("wg", (128, KT * DV), BF, kind="ExternalInput")
    wo_d = nc.dram_tensor("wo", (128, 2 * D), BF, kind="ExternalInput")
    bz_d = nc.dram_tensor("bz", (DK, 1), F32, kind="ExternalInput")
    out_d = nc.dram_tensor("out", (T, D), BF, kind="ExternalOutput")

    with tile.TileContext(nc) as tc:
        with (
            tc.tile_pool(name="consts", bufs=1) as consts,
            tc.tile_pool(name="wpool", bufs=1) as wpool,
            tc.tile_pool(name="persist", bufs=1) as pers,
            tc.tile_pool(name="small", bufs=4) as small,
            tc.tile_pool(name="xt", bufs=2) as xtp,
            tc.tile_pool(name="stage", bufs=3) as stage,
            tc.tile_pool(name="ps_proj", bufs=3, space="PSUM") as psp,
            tc.tile_pool(name="ps_pa", bufs=1, space="PSUM") as pspa,
            tc.tile_pool(name="ps_po", bufs=1, space="PSUM") as pspo,
            tc.tile_pool(name="ps_misc", bufs=1, space="PSUM") as psmisc,
            tc.tile_pool(name="ps_out", bufs=2, space="PSUM") as psout,
        ):
            # ---- constants (scalar ring so they don't delay x/weights) ----
            jj = np.arange(128)[:, None]
            ii = np.arange(SC)[None, :]
            m0_np = (jj <= ii).astype(np.float32)                    # [128,256]
            m1_np = (jj + 128 <= ii[:, 128:]).astype(np.float32)     # [128,128]
            m01_d = nc.inline_tensor(
                np.concatenate([m0_np, m1_np], axis=1), name="m01_c")
            ident_d = nc.inline_tensor(
                np.eye(128, dtype=np.float32).astype(ml_dtypes.bfloat16),
                name="ident_c")
            m01 = consts.tile([128, 384], F32)
            nc.scalar.dma_start(m01, m01_d[:, :])
            m0 = m01[:, 0:256]
            m1 = m01[:, 256:384]
            ident = consts.tile([128, 128], BF)
            nc.scalar.dma_start(ident, ident_d[:, :])
            bz_sb = consts.tile([128, 1], F32)
            nc.scalar.dma_start(bz_sb, bz_d[:, :])
            neginf = consts.tile([128, SC], F32)
            nc.vector.memset(neginf, -3.0e38)

            xt3 = xt_d.rearrange("(k p) t -> p k t", p=128)

            xqs = [xtp.tile([128, KT, 512], BF, tag="xq", name=f"xq{i}")
                   for i in range(4)]
            wq_sb = wpool.tile([128, KT, DK], BF)
            wk_sb = wpool.tile([128, KT, DK], BF)
            wz_sb = wpool.tile([128, KT, DK], BF)
            wv_sb = wpool.tile([128, KT, DV], BF)
            wg_sb = wpool.tile([128, KT, DV], BF)
            wo_sb = wpool.tile([128, 2, D], BF)
            nc.sync.dma_start(wz_sb, wz_d.rearrange("p (k n) -> p k n", k=KT))
            nc.sync.dma_start(xqs[0][:, 0:4, :], xt3[:, 0:4, 0:512])
            nc.sync.dma_start(xqs[0][:, 4:6, :], xt3[:, 4:6, 0:512])
            nc.sync.dma_start(xqs[0][:, 6:8, :], xt3[:, 6:8, 0:512])
            nc.sync.dma_start(wq_sb, wq_d.rearrange("p (k n) -> p k n", k=KT))
            nc.sync.dma_start(wk_sb, wk_d.rearrange("p (k n) -> p k n", k=KT))
            nc.sync.dma_start(wv_sb, wv_d.rearrange("p (k n) -> p k n", k=KT))
            nc.sync.dma_start(wg_sb, wg_d.rearrange("p (k n) -> p k n", k=KT))

            # ---- persistent activations ----
            qg = pers.tile([128, T], BF)       # q^T * exp(G) * scale
            kg = pers.tile([128, T], BF)       # k^T * exp(-G)
            spf = pers.tile([128, T], F32)     # softplus/cumsum, then exp(-G)
            egf = pers.tile([128, T], F32)     # exp(G)
            v2_sb = pers.tile([128, NSC, 512], BF)   # v, 2 t-tiles per SC
            sg2_sb = pers.tile([128, NSC, 512], BF)  # silu(g)
            kbar = pers.tile([128, NSC, 2, 128], BF)  # [t, sc, jt, dk]
            og_sb = pers.tile([128, TT, DV], BF)
            ogt = pers.tile([128, 2, T], BF)   # gated output transposed
            s_ab = pers.tile([128, 2, DV], BF)  # double-buffered state
            spl = pers.tile([128, NSC], F32)
            elast = pers.tile([128, NSC], F32)

            def outproj(s):
                """PE transposes of og(s) + output projection + store."""
                for it in range(2):
                    tt = s * 2 + it
                    xsl = slice(tt * 128, (tt + 1) * 128)
                    ptp = psmisc.tile([128, 2, 128], BF, tag="misc", name="ptp")
                    for k2 in range(2):
                        k2sl = slice(k2 * 128, (k2 + 1) * 128)
                        nc.tensor.transpose(ptp[:, k2, :],
                                            og_sb[:, tt, k2sl], ident)
                    nc.vector.tensor_copy(ogt[:, :, xsl], ptp)
                    for nb in range(2):
                        nsl = slice(nb * 512, nb * 512 + 512)
                        pout = psout.tile([128, 512], F32, tag="pout")
                        for k2 in range(2):
                            nc.tensor.matmul(
                                pout, ogt[:, k2, xsl], wo_sb[:, k2, nsl],
                                start=(k2 == 0), stop=(k2 == 1))
                        st = stage.tile([128, 512], BF, tag="st")
                        if nb == 0:
                            nc.vector.tensor_copy(st, pout)
                            nc.sync.dma_start(out_d[xsl, nsl], st)
                        else:
                            nc.scalar.copy(st, pout)
                            nc.scalar.dma_start(out_d[xsl, nsl], st)

            def rec(s):
                """Recurrence superchunk s; also emits outproj(s-1)."""
                ssl = slice(s * SC, (s + 1) * SC)
                s_cur = s_ab[:, s % 2, :]
                s_nxt = s_ab[:, (s + 1) % 2, :]
                # state update first: S_nxt = elast * S_cur + k_bar^T @ v
                # (the DVE update overlaps the pa/po PE work; skipped for the
                # final superchunk - never read)
                if s < NSC - 1:
                    pd_ = psmisc.tile([128, DV], F32, tag="misc", name="pd_")
                    nc.tensor.matmul(pd_, kbar[:, s, 0, :], v2_sb[:, s, 0:256],
                                     start=True, stop=False)
                    nc.tensor.matmul(pd_, kbar[:, s, 1, :],
                                     v2_sb[:, s, 256:512],
                                     start=False, stop=True)
                    if s == 0:
                        nc.vector.tensor_copy(s_nxt, pd_)
                    else:
                        nc.vector.scalar_tensor_tensor(
                            out=s_nxt, in0=s_cur, scalar=elast[:, s:s + 1],
                            in1=pd_, op0=AL.mult, op1=AL.add)
                # intra-chunk scores A^T[j, i]; jt=1 only needs i >= 128
                pa = pspa.tile([128, 512], F32, tag="pa")
                jsl0 = slice(s * SC, s * SC + 128)
                jsl1 = slice(s * SC + 128, s * SC + 256)
                nc.tensor.matmul(pa[:, 0:256], kg[:, jsl0], qg[:, ssl],
                                 start=True, stop=True)
                nc.tensor.matmul(pa[:, 384:512], kg[:, jsl1], qg[:, jsl1],
                                 start=True, stop=True)
                am = small.tile([128, 2, SC], BF, tag="am")
                nc.gpsimd.tensor_mul(am[:, 0, :], pa[:, 0:256], m0)
                nc.gpsimd.tensor_mul(am[:, 1, 128:256], pa[:, 384:512], m1)
                # previous superchunk's output projection covers the am / og
                # dependency latency with PE work
                if s > 0:
                    outproj(s - 1)
                # o = qg @ S_prev + tril(A) @ v   (S_prev is 0 for s == 0)
                po = pspo.tile([128, 512], F32, tag="po")
                for it in range(2):
                    osl = slice(it * DV, it * DV + DV)
                    isl = slice(s * SC + it * 128, s * SC + it * 128 + 128)
                    if s > 0:
                        nc.tensor.matmul(po[:, osl], qg[:, isl], s_cur,
                                         start=True, stop=False)
                    for jt in range(it + 1):
                        nc.tensor.matmul(
                            po[:, osl],
                            am[:, jt, it * 128:it * 128 + 128],
                            v2_sb[:, s, jt * 256:jt * 256 + 256],
                            start=(s == 0 and jt == 0), stop=(jt == it))
                # epilogue: rmsnorm * swish-gate (rsqrt via DVE pow ALU;
                # the sqrt(DV) factor is folded into wo on the host)
                for it in range(2):
                    tt = s * 2 + it
                    osl = slice(it * DV, it * DV + DV)
                    scr = small.tile([128, DV], F32, tag="scr")
                    ssq = small.tile([128, 1], F32, tag="ssq")
                    nc.vector.tensor_tensor_reduce(
                        scr, po[:, osl], po[:, osl], 1.0, DV * EPS,
                        AL.mult, AL.add, accum_out=ssq)
                    rstd = small.tile([128, 1], F32, tag="rstd")
                    nc.vector.tensor_scalar(rstd, ssq, -0.5, None, AL.pow)
                    nc.vector.scalar_tensor_tensor(
                        out=og_sb[:, tt, :], in0=po[:, osl], scalar=rstd,
                        in1=sg2_sb[:, s, osl], op0=AL.mult, op1=AL.mult)

            # ====== projections + recurrence, per T-quarter of 512 ======
            for q4 in range(4):
                tsl = slice(q4 * 512, (q4 + 1) * 512)
                xq = xqs[q4]
                if q4 + 1 < 4:
                    nsl4 = slice((q4 + 1) * 512, (q4 + 2) * 512)
                    nc.sync.dma_start(xqs[q4 + 1][:, 0:4, :],
                                      xt3[:, 0:4, nsl4])
                    nc.sync.dma_start(xqs[q4 + 1][:, 4:8, :],
                                      xt3[:, 4:8, nsl4])
                if q4 == 0:
                    nc.sync.dma_start(
                        wo_sb, wo_d.rearrange("p (k n) -> p k n", k=2))
                # gate path (z) first - it feeds the longest chain
                pz = psp.tile([128, 512], F32, tag="pp")
                for k in range(KT):
                    nc.tensor.matmul(pz, wz_sb[:, k, :], xq[:, k, :],
                                     start=(k == 0), stop=(k == KT - 1))
                # sp = softplus(-(z + b)) = ln(1 + exp(-(z + b)))
                sp = spf[:, tsl]
                eg = egf[:, tsl]
                nc.scalar.activation(sp, pz, ACT.Exp, bias=bz_sb, scale=-1.0)
                nc.scalar.activation(sp, sp, ACT.Ln, bias=1.0)
                # per-superchunk cumsum of softplus
                for i2 in range(2):
                    lsl = slice(i2 * SC, (i2 + 1) * SC)
                    nc.vector.tensor_tensor_scan(
                        sp[:, lsl], sp[:, lsl], neginf, 0.0, AL.add, AL.max)
                # SP at superchunk ends, decay factors
                sp3 = sp.rearrange("p (s c) -> p s c", c=SC)
                nc.scalar.copy(
                    spl[:, 2 * q4:2 * q4 + 2].rearrange("p (s o) -> p s o", o=1),
                    sp3[:, 0:2, SC - 1:SC])
                nc.scalar.activation(elast[:, 2 * q4:2 * q4 + 2],
                                     spl[:, 2 * q4:2 * q4 + 2],
                                     ACT.Exp, scale=-1.0 / NORM)
                # eg = exp(G); exp(-G) = 1/eg on DVE (keeps ACT table fixed)
                nc.scalar.activation(eg, sp, ACT.Exp, scale=-1.0 / NORM)
                nc.vector.reciprocal(sp, eg)
                # qg = q^T * exp(G); kg = k^T * exp(-G)  (direct from PSUM)
                pq = psp.tile([128, 512], F32, tag="pp")
                for k in range(KT):
                    nc.tensor.matmul(pq, wq_sb[:, k, :], xq[:, k, :],
                                     start=(k == 0), stop=(k == KT - 1))
                nc.vector.tensor_mul(qg[:, tsl], pq, eg)
                pk = psp.tile([128, 512], F32, tag="pp")
                for k in range(KT):
                    nc.tensor.matmul(pk, wk_sb[:, k, :], xq[:, k, :],
                                     start=(k == 0), stop=(k == KT - 1))
                nc.vector.tensor_mul(kg[:, tsl], pk, sp)
                # k_bar^T = (kg^T * elast) transposed to [t, dk] on the PE
                # (bf16 transpose = 1 cyc/row; DMA-transpose would queue
                # behind the input loads on the serial DMA device)
                for s in (2 * q4, 2 * q4 + 1):
                    if s == NSC - 1:
                        continue
                    ssl = slice(s * SC, (s + 1) * SC)
                    kbt = small.tile([128, SC], BF, tag="kbt")
                    nc.vector.tensor_scalar_mul(kbt, kg[:, ssl],
                                                elast[:, s:s + 1])
                    ptp = psmisc.tile([128, 2, 128], BF, tag="misc", name="ptp")
                    for k2 in range(2):
                        k2sl = slice(k2 * 128, (k2 + 1) * 128)
                        nc.tensor.transpose(ptp[:, k2, :], kbt[:, k2sl], ident)
                    nc.vector.tensor_copy(kbar[:, s, :, :], ptp)
                # v, g in normal layout, two t-tiles at a time; the
                # recurrence superchunk interleaves between pair blocks so
                # its Pool/DVE chains hide under projection PE work
                def vg_pair(pair):
                    it0 = (pair % 2) * 2
                    pv = psp.tile([128, 512], F32, tag="pp")
                    for half in range(2):
                        xsl = slice((it0 + half) * 128, (it0 + half + 1) * 128)
                        osl = slice(half * 256, (half + 1) * 256)
                        for k in range(KT):
                            nc.tensor.matmul(pv[:, osl], xq[:, k, xsl],
                                             wv_sb[:, k, :],
                                             start=(k == 0), stop=(k == KT - 1))
                    nc.vector.tensor_copy(v2_sb[:, pair, :], pv)
                    pg = psp.tile([128, 512], F32, tag="pp")
                    for half in range(2):
                        xsl = slice((it0 + half) * 128, (it0 + half + 1) * 128)
                        osl = slice(half * 256, (half + 1) * 256)
                        for k in range(KT):
                            nc.tensor.matmul(pg[:, osl], xq[:, k, xsl],
                                             wg_sb[:, k, :],
                                             start=(k == 0), stop=(k == KT - 1))
                    # silu(g) = g * (1 + exp(-g))^-1 ; stays in Exp/Ln table
                    sgs = small.tile([128, 512], F32, tag="sgs")
                    nc.scalar.activation(sgs, pg, ACT.Exp, scale=-1.0)
                    nc.vector.tensor_scalar(sgs, sgs, 1.0, -1.0, AL.add, AL.pow)
                    nc.gpsimd.scalar_tensor_tensor(
                        out=sg2_sb[:, pair, :], in0=sgs, scalar=1.0, in1=pg,
                        op0=AL.mult, op1=AL.mult)

                vg_pair(2 * q4)
                rec(2 * q4)
                vg_pair(2 * q4 + 1)
                rec(2 * q4 + 1)
            outproj(NSC - 1)
    try:
        nc.finalize()
    finally:
        bacc.get_activation_tables = _saved_fn
    return nc


def _get_nc():
    if "nc" not in _CACHE:
        _CACHE["nc"] = _build_program()
    return _CACHE["nc"]


def _sb_layout(w, kt=KT):
    """[kt*128, N] -> [128, kt*N] matching the SBUF [p, k, n] tile layout."""
    n = w.shape[1]
    return np.ascontiguousarray(
        w.reshape(kt, 128, n).transpose(1, 0, 2).reshape(128, kt * n))


def _make_in_maps(x, Wq, Wk, Wv, Wg, Wgk1, Wgk2, bgk2, gnorm_w, Wo):
    f = np.float32
    bf = ml_dtypes.bfloat16
    x = np.asarray(x, f)
    Wq = np.asarray(Wq, f)
    Wk = np.asarray(Wk, f)
    Wv = np.asarray(Wv, f)
    Wg = np.asarray(Wg, f)
    Wgk1 = np.asarray(Wgk1, f)
    Wgk2 = np.asarray(Wgk2, f)
    bgk2 = np.asarray(bgk2, f)
    gnorm_w = np.asarray(gnorm_w, f)
    Wo = np.asarray(Wo, f)

    scale = f(DK) ** f(-0.5)
    wz_full = Wgk1 @ Wgk2                      # [D, KD] fused low-rank gate proj
    in_maps = []
    for c in range(8):
        b, h = c // 4, c % 4
        kd = slice(h * DK, (h + 1) * DK)
        vd = slice(h * DV, (h + 1) * DV)
        # sqrt(DV)=16 from the rmsnorm rsqrt folding lives in wo
        wo = Wo[vd, :] * gnorm_w[:, None] * 16.0
        in_maps.append({
            "xt": np.ascontiguousarray(x[b].T).astype(bf),
            "wq": _sb_layout(Wq[:, kd] * scale).astype(bf),
            "wk": _sb_layout(Wk[:, kd]).astype(bf),
            "wz": _sb_layout(wz_full[:, kd]).astype(bf),
            "wv": _sb_layout(Wv[:, vd]).astype(bf),
            "wg": _sb_layout(Wg[:, vd]).astype(bf),
            "wo": _sb_layout(wo, kt=2).astype(bf),
            "bz": np.ascontiguousarray(-bgk2[kd]).reshape(DK, 1),
        })
    return in_maps


def _run(in_maps, **kwargs):
    from concourse.bass_utils import run_bass_kernel_spmd
    nc = _get_nc()
    return run_bass_kernel_spmd(nc, in_maps, core_ids=list(range(8)), **kwargs)


def _get_exec():
    """Build (once) a reusable 8-core jitted executable around the Bass NEFF.

    Mirrors bass2jax.run_bass_via_pjrt's multi-core path but without buffer
    donation, so repeat kernel() calls reuse the compiled executable instead
    of re-tracing and re-compiling.
    """
    if "exec" in _CACHE:
        return _CACHE["exec"]
    import jax
    import numpy as _np
    from jax.sharding import Mesh, PartitionSpec
    from jax.experimental.shard_map import shard_map
    import concourse.mybir as mybir
    from concourse import bass2jax
    from concourse.bass2jax import _bass_exec_p, partition_id_tensor

    nc = _get_nc()
    n_cores = 8
    bass2jax.install_neuronx_cc_hook()
    partition_name = nc.partition_id_tensor.name if nc.partition_id_tensor else None
    in_names, out_names, out_avals, zero_outs = [], [], [], []
    for alloc in nc.m.functions[0].allocations:
        if not isinstance(alloc, mybir.MemoryLocationSet):
            continue
        name = alloc.memorylocations[0].name
        if alloc.kind == "ExternalInput":
            if name != partition_name:
                in_names.append(name)
        elif alloc.kind == "ExternalOutput":
            out_names.append(name)
            shape = tuple(alloc.tensor_shape)
            dtype = mybir.dt.np(alloc.dtype)
            out_avals.append(jax.core.ShapedArray(shape, dtype))
            zero_outs.append(_np.zeros(shape, dtype))
    n_params = len(in_names)
    all_in_names = list(in_names) + out_names
    if partition_name is not None:
        all_in_names.append(partition_name)

    def _body(*args):
        operands = list(args)
        if partition_name is not None:
            operands.append(partition_id_tensor())
        outs = _bass_exec_p.bind(
            *operands,
            out_avals=tuple(out_avals),
            in_names=tuple(all_in_names),
            out_names=tuple(out_names),
            lowering_input_output_aliases=(),
            sim_require_finite=True,
            sim_require_nnan=True,
            nc=nc,
        )
        return tuple(outs)

    devices = jax.devices()[:n_cores]
    mesh = Mesh(_np.asarray(devices), ("core",))
    in_specs = (PartitionSpec("core"),) * (n_params + len(out_names))
    out_specs = (PartitionSpec("core"),) * len(out_names)
    fn = jax.jit(shard_map(_body, mesh=mesh, in_specs=in_specs,
                           out_specs=out_specs, check_rep=False),
                 keep_unused=True)
    concat_zero = [
        _np.zeros((n_cores * z.shape[0],) + z.shape[1:], z.dtype)
        for z in zero_outs
    ]
    _CACHE["exec"] = (fn, in_names, out_names, concat_zero, n_cores)
    return _CACHE["exec"]


def kernel(x, Wq, Wk, Wv, Wg, Wgk1, Wgk2, bgk2, gnorm_w, Wo):
    import jax
    in_maps = _make_in_maps(x, Wq, Wk, Wv, Wg, Wgk1, Wgk2, bgk2, gnorm_w, Wo)
    fn, in_names, out_names, concat_zero, n_cores = _get_exec()
    concat_in = [
        np.concatenate([np.asarray(in_maps[c][nm]) for c in range(n_cores)],
                       axis=0)
        for nm in in_names
    ]
    outs = fn(*concat_in, *concat_zero)
    jax.block_until_ready(outs)
    o = np.asarray(outs[out_names.index("out")]).reshape(n_cores, T, D)
    o = o.astype(np.float32)
    out = np.zeros((B, T, D), np.float32)
    for c in range(n_cores):
        out[c // 4] += o[c]
    return out


# revision 18
# speedup vs baseline: 1.2304x; 1.1362x over previous
"""FLA gated linear attention (chunked) for Trainium2, 8-core SPMD.

Sharding: 8 cores = B(2) x H(4); each core handles one (batch, head) pair:
  - head-sliced q/k/v/g projections + low-rank gate projection (fused on host
    into one [D, DK] matrix),
  - chunked gated linear attention recurrence (superchunks of 256 positions),
  - fused RMSNorm * swish gate,
  - row-parallel output projection producing a [T, D] partial; host sums the
    4 head-partials per batch.

bf16 compute pipeline: all matmul operands bf16 (PE rate identical to fp32r
but DMA bytes halve and DVE ops speed up); gate log-space chain (softplus /
cumsum / exp) stays fp32. k_bar transposes ride the XBAR DMA-transpose unit
(latency hidden by the projection phase); o transposes stay on the PE where
latency is nil. rsqrt and sigmoid avoid ACT-table switches via the DVE pow
ALU. The recurrence is interleaved into the projection loop (superchunks 2q,
2q+1 run right after quarter q) with the output projection lagged one
superchunk so eplilogue latency hides under PE work. Output partials are
stored bf16 and upcast+summed on host.

Self-contained: hardcodes all shapes; host-side work is only sharding/layout.
"""
import sys
sys.path.insert(0, "/opt/trn_rl_repo")

import numpy as np
import ml_dtypes

B, T, D = 2, 2048, 1024
H = 4
DK, DV = 128, 256
SC, NSC = 256, 8          # superchunk size / count
KT = 8                    # 128-row k-tiles over D
TT = 16                   # 128-row t-tiles over T
NORM = 16.0               # gate logit normalizer
EPS = 1e-6

_CACHE = {}


def _build_program():
    import concourse.tile as tile
    from concourse import bacc, hw_specs, mybir

    # Collapse the ACT piecewise-table choice to a single combined
    # Ln+Exp+Copy set so the scheduler never inserts act-table reloads.
    _keep = {"natural_log_exp_and_others"}
    _orig_tables = hw_specs.get_activation_tables("gen3")
    _filtered = {n: (s if n in _keep else set()) for n, s in _orig_tables.items()}
    _saved_fn = bacc.get_activation_tables
    bacc.get_activation_tables = lambda arch: _filtered

    BF = mybir.dt.bfloat16
    F32 = mybir.dt.float32
    F8 = mybir.dt.float8e4
    AL = mybir.AluOpType
    ACT = mybir.ActivationFunctionType
    DR = mybir.MatmulPerfMode.DoubleRow

    nc = bacc.Bacc()

    # x and the five projection weights ship as fp8e4m3 hi+lo pairs; the
    # three-term compensated DoubleRow matmul (xh@wh + xl@wh + xh@wl) runs
    # at 0.75x the bf16 PE cost with ~bf16 accuracy. Host pre-scales the
    # operands (x16 / x128) so the lo residuals stay in e4m3 normal range;
    # the inverse scales fold into existing post-matmul scalar ops.
    xth_d = nc.dram_tensor("xth", (D, T), F8, kind="ExternalInput")
    xtl_d = nc.dram_tensor("xtl", (D, T), F8, kind="ExternalInput")
    wqh_d = nc.dram_tensor("wqh", (128, KT * DK), F8, kind="ExternalInput")
    wql_d = nc.dram_tensor("wql", (128, KT * DK), F8, kind="ExternalInput")
    wkh_d = nc.dram_tensor("wkh", (128, KT * DK), F8, kind="ExternalInput")
    wkl_d = nc.dram_tensor("wkl", (128, KT * DK), F8, kind="ExternalInput")
    wzh_d = nc.dram_tensor("wzh", (128, KT * DK), F8, kind="ExternalInput")
    wzl_d = nc.dram_tensor("wzl", (128, KT * DK), F8, kind="ExternalInput")
    wvh_d = nc.dram_tensor("wvh", (128, KT * DV), F8, kind="ExternalInput")
    wvl_d = nc.dram_tensor("wvl", (128, KT * DV), F8, kind="ExternalInput")
    wgh_d = nc.dram_tensor("wgh", (128, KT * DV), F8, kind="ExternalInput")
    wgl_d = nc.dram_tensor("wgl", (128, KT * DV), F8, kind="ExternalInput")
    wo_d = nc.dram_tensor("wo", (128, 2 * D), BF, kind="ExternalInput")
    bz_d = nc.dram_tensor("bz", (DK, 1), F32, kind="ExternalInput")
    out_d = nc.dram_tensor("out", (T, D), BF, kind="ExternalOutput")

    CX = 16.0                  # host pre-scale on x
    CWQ = 128.0                # host pre-scale on wq (includes attn scale)
    CW = 16.0                  # host pre-scale on wk/wz/wv/wg
    CQ = 1.0 / (CX * CWQ)
    CZ = 1.0 / (CX * CW)       # = CK = CV = CG

    with tile.TileContext(nc) as tc:
        with (
            tc.tile_pool(name="consts", bufs=1) as consts,
            tc.tile_pool(name="wpool", bufs=1) as wpool,
            tc.tile_pool(name="persist", bufs=1) as pers,
            tc.tile_pool(name="small", bufs=4) as small,
            tc.tile_pool(name="xt", bufs=2) as xtp,
            tc.tile_pool(name="stage", bufs=3) as stage,
            tc.tile_pool(name="ps_proj", bufs=3, space="PSUM") as psp,
            tc.tile_pool(name="ps_pa", bufs=1, space="PSUM") as pspa,
            tc.tile_pool(name="ps_po", bufs=1, space="PSUM") as pspo,
            tc.tile_pool(name="ps_misc", bufs=1, space="PSUM") as psmisc,
            tc.tile_pool(name="ps_out", bufs=2, space="PSUM") as psout,
        ):
            # ---- constants (scalar ring so they don't delay x/weights) ----
            jj = np.arange(128)[:, None]
            ii = np.arange(SC)[None, :]
            m0_np = (jj <= ii).astype(np.float32)                    # [128,256]
            m1_np = (jj + 128 <= ii[:, 128:]).astype(np.float32)     # [128,128]
            m01_d = nc.inline_tensor(
                np.concatenate([m0_np, m1_np], axis=1), name="m01_c")
            ident_d = nc.inline_tensor(
                np.eye(128, dtype=np.float32).astype(ml_dtypes.bfloat16),
                name="ident_c")
            m01 = consts.tile([128, 384], F32)
            nc.scalar.dma_start(m01, m01_d[:, :])
            m0 = m01[:, 0:256]
            m1 = m01[:, 256:384]
            ident = consts.tile([128, 128], BF)
            nc.scalar.dma_start(ident, ident_d[:, :])
            bz_sb = consts.tile([128, 1], F32)
            nc.scalar.dma_start(bz_sb, bz_d[:, :])
            neginf = consts.tile([128, SC], F32)
            nc.vector.memset(neginf, -3.0e38)

            xh3 = xth_d.rearrange("(k p) t -> p k t", p=128)
            xl3 = xtl_d.rearrange("(k p) t -> p k t", p=128)

            xqhs = [xtp.tile([128, KT, 512], F8, tag="xqh", name=f"xqh{i}")
                    for i in range(4)]
            xqls = [xtp.tile([128, KT, 512], F8, tag="xql", name=f"xql{i}")
                    for i in range(4)]
            wqh_sb = wpool.tile([128, KT, DK], F8)
            wql_sb = wpool.tile([128, KT, DK], F8)
            wkh_sb = wpool.tile([128, KT, DK], F8)
            wkl_sb = wpool.tile([128, KT, DK], F8)
            wzh_sb = wpool.tile([128, KT, DK], F8)
            wzl_sb = wpool.tile([128, KT, DK], F8)
            wvh_sb = wpool.tile([128, KT, DV], F8)
            wvl_sb = wpool.tile([128, KT, DV], F8)
            wgh_sb = wpool.tile([128, KT, DV], F8)
            wgl_sb = wpool.tile([128, KT, DV], F8)
            wo_sb = wpool.tile([128, 2, D], BF)
            rr = lambda d, kt=KT: d.rearrange("p (k n) -> p k n", k=kt)
            nc.sync.dma_start(wzh_sb, rr(wzh_d))
            nc.sync.dma_start(xqhs[0][:, 0:4, :], xh3[:, 0:4, 0:512])
            nc.sync.dma_start(wzl_sb, rr(wzl_d))
            nc.sync.dma_start(xqls[0][:, 0:4, :], xl3[:, 0:4, 0:512])
            nc.sync.dma_start(xqhs[0][:, 4:8, :], xh3[:, 4:8, 0:512])
            nc.sync.dma_start(xqls[0][:, 4:8, :], xl3[:, 4:8, 0:512])
            nc.sync.dma_start(wqh_sb, rr(wqh_d))
            nc.sync.dma_start(wkh_sb, rr(wkh_d))
            nc.sync.dma_start(wvh_sb, rr(wvh_d))
            nc.sync.dma_start(wgh_sb, rr(wgh_d))
            nc.sync.dma_start(wql_sb, rr(wql_d))
            nc.sync.dma_start(wkl_sb, rr(wkl_d))
            nc.sync.dma_start(wvl_sb, rr(wvl_d))
            nc.sync.dma_start(wgl_sb, rr(wgl_d))

            def proj_t(pp, wh, wl, xh, xl):
                """Transposed-layout projection: pp = (xh+xl)^T (wh+wl),
                three compensated DoubleRow terms over k-tile pairs."""
                n = 0
                for w8, x8 in ((wh, xh), (wl, xh), (wh, xl)):
                    for i in range(4):
                        n += 1
                        nc.tensor.matmul(
                            pp, w8[:, 2 * i:2 * i + 2, :],
                            x8[:, 2 * i:2 * i + 2, :],
                            start=(n == 1), stop=(n == 12), perf_mode=DR)

            def proj_n(pp, osl, xsl, xh, xl, wh, wl):
                """Normal-layout projection tile."""
                n = 0
                for x8, w8 in ((xh, wh), (xl, wh), (xh, wl)):
                    for i in range(4):
                        n += 1
                        nc.tensor.matmul(
                            pp[:, osl], x8[:, 2 * i:2 * i + 2, xsl],
                            w8[:, 2 * i:2 * i + 2, :],
                            start=(n == 1), stop=(n == 12), perf_mode=DR)

            # ---- persistent activations ----
            qg = pers.tile([128, T], BF)       # q^T * exp(G) * scale
            kg = pers.tile([128, T], BF)       # k^T * exp(-G)
            spf = pers.tile([128, T], F32)     # softplus/cumsum, then exp(-G)
            egf = pers.tile([128, T], F32)     # exp(G)
            v2_sb = pers.tile([128, NSC, 512], BF)   # v, 2 t-tiles per SC
            sg2_sb = pers.tile([128, NSC, 512], BF)  # silu(g)
            kbar = pers.tile([128, NSC, 2, 128], BF)  # [t, sc, jt, dk]
            og_sb = pers.tile([128, TT, DV], BF)
            ogt = pers.tile([128, 2, T], BF)   # gated output transposed
            s_ab = pers.tile([128, 2, DV], BF)  # double-buffered state
            spl = pers.tile([128, NSC], F32)
            elast = pers.tile([128, NSC], F32)

            def outproj(s):
                """PE transposes of og(s) + output projection + store."""
                for it in range(2):
                    tt = s * 2 + it
                    xsl = slice(tt * 128, (tt + 1) * 128)
                    ptp = psmisc.tile([128, 2, 128], BF, tag="misc", name="ptp")
                    for k2 in range(2):
                        k2sl = slice(k2 * 128, (k2 + 1) * 128)
                        nc.tensor.transpose(ptp[:, k2, :],
                                            og_sb[:, tt, k2sl], ident)
                    nc.vector.tensor_copy(ogt[:, :, xsl], ptp)
                    for nb in range(2):
                        nsl = slice(nb * 512, nb * 512 + 512)
                        pout = psout.tile([128, 512], F32, tag="pout")
                        for k2 in range(2):
                            nc.tensor.matmul(
                                pout, ogt[:, k2, xsl], wo_sb[:, k2, nsl],
                                start=(k2 == 0), stop=(k2 == 1))
                        st = stage.tile([128, 512], BF, tag="st")
                        if nb == 0:
                            nc.vector.tensor_copy(st, pout)
                            nc.sync.dma_start(out_d[xsl, nsl], st)
                        else:
                            nc.scalar.copy(st, pout)
                            nc.scalar.dma_start(out_d[xsl, nsl], st)

            def rec(s):
                """Recurrence superchunk s; also emits outproj(s-1)."""
                ssl = slice(s * SC, (s + 1) * SC)
                s_cur = s_ab[:, s % 2, :]
                s_nxt = s_ab[:, (s + 1) % 2, :]
                # state update first: S_nxt = elast * S_cur + k_bar^T @ v
                # (the DVE update overlaps the pa/po PE work; skipped for the
                # final superchunk - never read)
                if s < NSC - 1:
                    pd_ = psmisc.tile([128, DV], F32, tag="misc", name="pd_")
                    nc.tensor.matmul(pd_, kbar[:, s, 0, :], v2_sb[:, s, 0:256],
                                     start=True, stop=False)
                    nc.tensor.matmul(pd_, kbar[:, s, 1, :],
                                     v2_sb[:, s, 256:512],
                                     start=False, stop=True)
                    if s == 0:
                        nc.vector.tensor_copy(s_nxt, pd_)
                    else:
                        nc.vector.scalar_tensor_tensor(
                            out=s_nxt, in0=s_cur, scalar=elast[:, s:s + 1],
                            in1=pd_, op0=AL.mult, op1=AL.add)
                # intra-chunk scores A^T[j, i]; jt=1 only needs i >= 128
                pa = pspa.tile([128, 512], F32, tag="pa")
                jsl0 = slice(s * SC, s * SC + 128)
                jsl1 = slice(s * SC + 128, s * SC + 256)
                nc.tensor.matmul(pa[:, 0:256], kg[:, jsl0], qg[:, ssl],
                                 start=True, stop=True)
                nc.tensor.matmul(pa[:, 384:512], kg[:, jsl1], qg[:, jsl1],
                                 start=True, stop=True)
                am = small.tile([128, 2, SC], BF, tag="am")
                nc.gpsimd.tensor_mul(am[:, 0, :], pa[:, 0:256], m0)
                nc.gpsimd.tensor_mul(am[:, 1, 128:256], pa[:, 384:512], m1)
                # previous superchunk's output projection covers the am / og
                # dependency latency with PE work
                if s > 0:
                    outproj(s - 1)
                # o = qg @ S_prev + tril(A) @ v   (S_prev is 0 for s == 0)
                po = pspo.tile([128, 512], F32, tag="po")
                for it in range(2):
                    osl = slice(it * DV, it * DV + DV)
                    isl = slice(s * SC + it * 128, s * SC + it * 128 + 128)
                    if s > 0:
                        nc.tensor.matmul(po[:, osl], qg[:, isl], s_cur,
                                         start=True, stop=False)
                    for jt in range(it + 1):
                        nc.tensor.matmul(
                            po[:, osl],
                            am[:, jt, it * 128:it * 128 + 128],
                            v2_sb[:, s, jt * 256:jt * 256 + 256],
                            start=(s == 0 and jt == 0), stop=(jt == it))
                # epilogue: rmsnorm * swish-gate (rsqrt via DVE pow ALU;
                # the sqrt(DV) factor is folded into wo on the host)
                for it in range(2):
                    tt = s * 2 + it
                    osl = slice(it * DV, it * DV + DV)
                    scr = small.tile([128, DV], F32, tag="scr")
                    ssq = small.tile([128, 1], F32, tag="ssq")
                    nc.vector.tensor_tensor_reduce(
                        scr, po[:, osl], po[:, osl], 1.0, DV * EPS,
                        AL.mult, AL.add, accum_out=ssq)
                    rstd = small.tile([128, 1], F32, tag="rstd")
                    nc.vector.tensor_scalar(rstd, ssq, -0.5, None, AL.pow)
                    nc.vector.scalar_tensor_tensor(
                        out=og_sb[:, tt, :], in0=po[:, osl], scalar=rstd,
                        in1=sg2_sb[:, s, osl], op0=AL.mult, op1=AL.mult)

            # ====== projections + recurrence, per T-quarter of 512 ======
            for q4 in range(4):
                tsl = slice(q4 * 512, (q4 + 1) * 512)
                xqh = xqhs[q4]
                xql = xqls[q4]
                if q4 + 1 < 4:
                    nsl4 = slice((q4 + 1) * 512, (q4 + 2) * 512)
                    nc.sync.dma_start(xqhs[q4 + 1][:, :, :],
                                      xh3[:, :, nsl4])
                    nc.sync.dma_start(xqls[q4 + 1][:, :, :],
                                      xl3[:, :, nsl4])
                if q4 == 0:
                    nc.sync.dma_start(
                        wo_sb, wo_d.rearrange("p (k n) -> p k n", k=2))
                # gate path (z) first - it feeds the longest chain
                pz = psp.tile([128, 512], F32, tag="pp")
                proj_t(pz, wzh_sb, wzl_sb, xqh, xql)
                # sp = softplus(-(z + b)) = ln(1 + exp(-(z + b)))
                sp = spf[:, tsl]
                eg = egf[:, tsl]
                nc.scalar.activation(sp, pz, ACT.Exp, bias=bz_sb, scale=-CZ)
                nc.scalar.activation(sp, sp, ACT.Ln, bias=1.0)
                # per-superchunk cumsum of softplus
                for i2 in range(2):
                    lsl = slice(i2 * SC, (i2 + 1) * SC)
                    nc.vector.tensor_tensor_scan(
                        sp[:, lsl], sp[:, lsl], neginf, 0.0, AL.add, AL.max)
                # SP at superchunk ends, decay factors
                sp3 = sp.rearrange("p (s c) -> p s c", c=SC)
                nc.scalar.copy(
                    spl[:, 2 * q4:2 * q4 + 2].rearrange("p (s o) -> p s o", o=1),
                    sp3[:, 0:2, SC - 1:SC])
                nc.scalar.activation(elast[:, 2 * q4:2 * q4 + 2],
                                     spl[:, 2 * q4:2 * q4 + 2],
                                     ACT.Exp, scale=-1.0 / NORM)
                # eg = exp(G); exp(-G) = 1/eg on DVE (keeps ACT table fixed)
                nc.scalar.activation(eg, sp, ACT.Exp, scale=-1.0 / NORM)
                nc.vector.reciprocal(sp, eg)
                # qg = q^T * exp(G); kg = k^T * exp(-G)  (direct from PSUM,
                # fp8 pre-scale compensated in the scalar slot)
                pq = psp.tile([128, 512], F32, tag="pp")
                proj_t(pq, wqh_sb, wql_sb, xqh, xql)
                nc.vector.scalar_tensor_tensor(
                    out=qg[:, tsl], in0=pq, scalar=CQ, in1=eg,
                    op0=AL.mult, op1=AL.mult)
                pk = psp.tile([128, 512], F32, tag="pp")
                proj_t(pk, wkh_sb, wkl_sb, xqh, xql)
                nc.vector.scalar_tensor_tensor(
                    out=kg[:, tsl], in0=pk, scalar=CZ, in1=sp,
                    op0=AL.mult, op1=AL.mult)
                # k_bar^T = (kg^T * elast) transposed to [t, dk] on the PE
                # (bf16 transpose = 1 cyc/row; DMA-transpose would queue
                # behind the input loads on the serial DMA device)
                for s in (2 * q4, 2 * q4 + 1):
                    if s == NSC - 1:
                        continue
                    ssl = slice(s * SC, (s + 1) * SC)
                    kbt = small.tile([128, SC], BF, tag="kbt")
                    nc.vector.tensor_scalar_mul(kbt, kg[:, ssl],
                                                elast[:, s:s + 1])
                    ptp = psmisc.tile([128, 2, 128], BF, tag="misc", name="ptp")
                    for k2 in range(2):
                        k2sl = slice(k2 * 128, (k2 + 1) * 128)
                        nc.tensor.transpose(ptp[:, k2, :], kbt[:, k2sl], ident)
                    nc.vector.tensor_copy(kbar[:, s, :, :], ptp)
                # v, g in normal layout, two t-tiles at a time; the
                # recurrence superchunk interleaves between pair blocks so
                # its Pool/DVE chains hide under projection PE work
                def vg_pair(pair):
                    it0 = (pair % 2) * 2
                    pv = psp.tile([128, 512], F32, tag="pp")
                    for half in range(2):
                        xsl = slice((it0 + half) * 128, (it0 + half + 1) * 128)
                        osl = slice(half * 256, (half + 1) * 256)
                        proj_n(pv, osl, xsl, xqh, xql, wvh_sb, wvl_sb)
                    nc.vector.tensor_scalar_mul(v2_sb[:, pair, :], pv, CZ)
                    pg = psp.tile([128, 512], F32, tag="pp")
                    for half in range(2):
                        xsl = slice((it0 + half) * 128, (it0 + half + 1) * 128)
                        osl = slice(half * 256, (half + 1) * 256)
                        proj_n(pg, osl, xsl, xqh, xql, wgh_sb, wgl_sb)
                    # silu(g) = g * (1 + exp(-g))^-1 ; stays in Exp/Ln table
                    sgs = small.tile([128, 512], F32, tag="sgs")
                    nc.scalar.activation(sgs, pg, ACT.Exp, scale=-CZ)
                    nc.vector.tensor_scalar(sgs, sgs, 1.0, -1.0, AL.add, AL.pow)
                    nc.gpsimd.scalar_tensor_tensor(
                        out=sg2_sb[:, pair, :], in0=sgs, scalar=CZ, in1=pg,
                        op0=AL.mult, op1=AL.mult)

                vg_pair(2 * q4)
                rec(2 * q4)
                vg_pair(2 * q4 + 1)
                rec(2 * q4 + 1)
            outproj(NSC - 1)
    try:
        nc.finalize()
    finally:
        bacc.get_activation_tables = _saved_fn
    return nc


def _get_nc():
    if "nc" not in _CACHE:
        _CACHE["nc"] = _build_program()
    return _CACHE["nc"]


def _sb_layout(w, kt=KT):
    """[kt*128, N] -> [128, kt*N] matching the SBUF [p, k, n] tile layout."""
    n = w.shape[1]
    return np.ascontiguousarray(
        w.reshape(kt, 128, n).transpose(1, 0, 2).reshape(128, kt * n))


def _make_in_maps(x, Wq, Wk, Wv, Wg, Wgk1, Wgk2, bgk2, gnorm_w, Wo):
    f = np.float32
    bf = ml_dtypes.bfloat16
    x = np.asarray(x, f)
    Wq = np.asarray(Wq, f)
    Wk = np.asarray(Wk, f)
    Wv = np.asarray(Wv, f)
    Wg = np.asarray(Wg, f)
    Wgk1 = np.asarray(Wgk1, f)
    Wgk2 = np.asarray(Wgk2, f)
    bgk2 = np.asarray(bgk2, f)
    gnorm_w = np.asarray(gnorm_w, f)
    Wo = np.asarray(Wo, f)

    f8 = ml_dtypes.float8_e4m3fn

    def split8(a):
        h = np.ascontiguousarray(a).astype(f8)
        l = (a - h.astype(f)).astype(f8)
        return h, l

    scale = f(DK) ** f(-0.5)
    wz_full = Wgk1 @ Wgk2                      # [D, KD] fused low-rank gate proj
    in_maps = []
    for c in range(8):
        b, h = c // 4, c % 4
        kd = slice(h * DK, (h + 1) * DK)
        vd = slice(h * DV, (h + 1) * DV)
        # sqrt(DV)=16 from the rmsnorm rsqrt folding lives in wo
        wo = Wo[vd, :] * gnorm_w[:, None] * 16.0
        xth, xtl = split8(x[b].T * 16.0)
        wqh, wql = split8(_sb_layout(Wq[:, kd] * (scale * 128.0)))
        wkh, wkl = split8(_sb_layout(Wk[:, kd] * 16.0))
        wzh, wzl = split8(_sb_layout(wz_full[:, kd] * 16.0))
        wvh, wvl = split8(_sb_layout(Wv[:, vd] * 16.0))
        wgh, wgl = split8(_sb_layout(Wg[:, vd] * 16.0))
        in_maps.append({
            "xth": xth, "xtl": xtl,
            "wqh": wqh, "wql": wql,
            "wkh": wkh, "wkl": wkl,
            "wzh": wzh, "wzl": wzl,
            "wvh": wvh, "wvl": wvl,
            "wgh": wgh, "wgl": wgl,
            "wo": _sb_layout(wo, kt=2).astype(bf),
            "bz": np.ascontiguousarray(-bgk2[kd]).reshape(DK, 1),
        })
    return in_maps


def _run(in_maps, **kwargs):
    from concourse.bass_utils import run_bass_kernel_spmd
    nc = _get_nc()
    return run_bass_kernel_spmd(nc, in_maps, core_ids=list(range(8)), **kwargs)


def _get_exec():
    """Build (once) a reusable 8-core jitted executable around the Bass NEFF.

    Mirrors bass2jax.run_bass_via_pjrt's multi-core path but without buffer
    donation, so repeat kernel() calls reuse the compiled executable instead
    of re-tracing and re-compiling.
    """
    if "exec" in _CACHE:
        return _CACHE["exec"]
    import jax
    import numpy as _np
    from jax.sharding import Mesh, PartitionSpec
    from jax.experimental.shard_map import shard_map
    import concourse.mybir as mybir
    from concourse import bass2jax
    from concourse.bass2jax import _bass_exec_p, partition_id_tensor

    nc = _get_nc()
    n_cores = 8
    bass2jax.install_neuronx_cc_hook()
    partition_name = nc.partition_id_tensor.name if nc.partition_id_tensor else None
    in_names, out_names, out_avals, zero_outs = [], [], [], []
    for alloc in nc.m.functions[0].allocations:
        if not isinstance(alloc, mybir.MemoryLocationSet):
            continue
        name = alloc.memorylocations[0].name
        if alloc.kind == "ExternalInput":
            if name != partition_name:
                in_names.append(name)
        elif alloc.kind == "ExternalOutput":
            out_names.append(name)
            shape = tuple(alloc.tensor_shape)
            dtype = mybir.dt.np(alloc.dtype)
            out_avals.append(jax.core.ShapedArray(shape, dtype))
            zero_outs.append(_np.zeros(shape, dtype))
    n_params = len(in_names)
    all_in_names = list(in_names) + out_names
    if partition_name is not None:
        all_in_names.append(partition_name)

    def _body(*args):
        operands = list(args)
        if partition_name is not None:
            operands.append(partition_id_tensor())
        outs = _bass_exec_p.bind(
            *operands,
            out_avals=tuple(out_avals),
            in_names=tuple(all_in_names),
            out_names=tuple(out_names),
            lowering_input_output_aliases=(),
            sim_require_finite=True,
            sim_require_nnan=True,
            nc=nc,
        )
        return tuple(outs)

    devices = jax.devices()[:n_cores]
    mesh = Mesh(_np.asarray(devices), ("core",))
    in_specs = (PartitionSpec("core"),) * (n_params + len(out_names))
    out_specs = (PartitionSpec("core"),) * len(out_names)
    fn = jax.jit(shard_map(_body, mesh=mesh, in_specs=in_specs,
                           out_specs=out_specs, check_rep=False),
                 keep_unused=True)
    concat_zero = [
        _np.zeros((n_cores * z.shape[0],) + z.shape[1:], z.dtype)
        for z in zero_outs
    ]
    _CACHE["exec"] = (fn, in_names, out_names, concat_zero, n_cores)
    return _CACHE["exec"]


def kernel(x, Wq, Wk, Wv, Wg, Wgk1, Wgk2, bgk2, gnorm_w, Wo):
    import jax
    in_maps = _make_in_maps(x, Wq, Wk, Wv, Wg, Wgk1, Wgk2, bgk2, gnorm_w, Wo)
    fn, in_names, out_names, concat_zero, n_cores = _get_exec()
    concat_in = [
        np.concatenate([np.asarray(in_maps[c][nm]) for c in range(n_cores)],
                       axis=0)
        for nm in in_names
    ]
    outs = fn(*concat_in, *concat_zero)
    jax.block_until_ready(outs)
    o = np.asarray(outs[out_names.index("out")]).reshape(n_cores, T, D)
    o = o.astype(np.float32)
    out = np.zeros((B, T, D), np.float32)
    for c in range(n_cores):
        out[c // 4] += o[c]
    return out
